# revision 77
# baseline (speedup 1.0000x reference)
"""Distributed Trainium2 kernel for nn_Attention_33002528702591.

Multi-head causal attention with RoPE (B=2, S=2048, D=2048, H=16, HD=128),
run across 8 NeuronCores with a hybrid data/tensor-parallel sharding:
core i handles batch (i // 4) and head group (i % 4) of 4 heads.

The four big GEMM groups (Q/K/V projections and the output projection) run
as fp8e4 DoubleRow matmuls with a 3-term error-compensation split: each
operand a is sent as a1 = fp8(s*a), a2 = fp8(s*a - a1) and the product is
a1@b1 + a1@b2 + a2@b1 (the a2@b2 term is ~(2.6%)^2 and dropped).  DoubleRow
contracts two 128-row k-chunks per instruction at 0.5 cycles/row, so the
3-term chain costs 0.75x the bf16 chain while keeping better-than-bf16
accuracy.  Scales are powers of two (x:8, w:64, ao:32) folded into csq/csk
(Q/K), the V eviction (1/512), the colsum ones vector (1/32 -> recip carries
x32), and the output eviction (1/2048), so they never round.  Scores and PV
stay bf16: their contraction is HD=128 / key-chunks whose 3-term split costs
as much as bf16, and naive fp8 error there exceeds the tolerance.

Each core computes, for its batch b and its 4 heads:
    QT = (wq_p @ x_b.T)   [512f, S]   (RoPE'd, pre-scaled by 1/sqrt(HD))
    KT = (wk_p @ x_b.T)   [512f, S]   (RoPE'd)
    V  = (x_b @ wv.T)     [S, 512f]
    per head h, q-tile: ST[k,q] = KT_h.T-chunks @ QT_h  (scores, transposed)
                        E = exp(ST) * causal_mask
                        colsum via E-stationary matmuls with a moving
                        ones[128,1] (engine cost ~1 cycle per chunk)
                        outT[hd,q] = sum_k V_chunk.T @ E;  outT *= 1/colsum
    partial[dout, t] = woT_slice.T @ attnoutT        [D, S]  (bf16)
The host sums the 4 per-batch partials and transposes back - that is the
"unshard" step for the row-parallel output projection.

Schedule: the attention chunk loop is software-pipelined (the scores matmul
for chunk k+1 is emitted before PV of chunk k, so the in-order TensorEngine
queue is never parked behind the ACT exp), and pure-PE filler matmuls are
woven between chunks: the NEXT tile's V projection during tile 0's
attention, the PREVIOUS tile's output projection during tiles 1-3's.
Diagonal chunks split PV into open + masked parts so only the 128-column
masked block waits on the Pool mask-multiply.

No device collectives are needed.  Layout trick: everything is kept
"feature-on-partition, token-on-free", with x / weights fed pre-transposed
and pre-split from the host; RoPE pairs are made contiguous by permuting
wq/wk ROWS on the host.
"""

import sys
from contextlib import ExitStack

import numpy as np

if "/opt/trn_rl_repo" not in sys.path:
    sys.path.insert(0, "/opt/trn_rl_repo")

import concourse.bass as bass
import concourse.tile as tile
from concourse import bacc, mybir

F32 = mybir.dt.float32
BF16 = mybir.dt.bfloat16
F8 = mybir.dt.float8e4
DR = mybir.MatmulPerfMode.DoubleRow

# problem constants
DIM = 2048
SEQ = 2048
BATCH = 2
N_HEADS = 16
HEAD_DIM = 128
N_CORES = 8
HEADS_PER_CORE = 4  # 2 batches x 4 head-groups = 8 cores

# fp8 scale plan (all powers of two; see module docstring)
SX = 8.0       # x pre-scale
SW = 64.0      # weight pre-scale
SAO = 32.0     # attention-out pre-scale
V_DESCALE = 1.0 / (SX * SW)          # V eviction
AO_RECIP = SAO                       # folded via ones_col = 1/SAO
OUT_DESCALE = 1.0 / (SAO * SW)       # output eviction


def build_graph(D=DIM, S=SEQ, HC=HEADS_PER_CORE, out_dtype=BF16):
    """One SPMD graph; per-core behavior differs only via input data."""
    HD = HEAD_DIM
    F = HC * HD            # features on this core (512)
    ND = D // 128          # d-chunks (16)
    NP = ND // 2           # d-chunk pairs (8) per DoubleRow contraction
    NT = S // 512          # token tiles (4)
    NF = F // 128          # feature tiles == heads (4)
    DQT = 512              # q tile width

    nc = bacc.Bacc()
    x1T = nc.declare_dram_parameter("x1T", [D, S], F8, False)
    x2T = nc.declare_dram_parameter("x2T", [D, S], F8, False)
    wq1T = nc.declare_dram_parameter("wq1T", [D, F], F8, False)
    wq2T = nc.declare_dram_parameter("wq2T", [D, F], F8, False)
    wk1T = nc.declare_dram_parameter("wk1T", [D, F], F8, False)
    wk2T = nc.declare_dram_parameter("wk2T", [D, F], F8, False)
    wv1T = nc.declare_dram_parameter("wv1T", [D, F], F8, False)
    wv2T = nc.declare_dram_parameter("wv2T", [D, F], F8, False)
    wo1T = nc.declare_dram_parameter("wo1T", [F, D], F8, False)
    wo2T = nc.declare_dram_parameter("wo2T", [F, D], F8, False)
    # [128, 2S]: rows 0:64 = Q rope (cos cols [0:S], sin cols [S:2S]),
    # rows 64:128 = K rope.  Vector TensorTensor requires equal SBUF base
    # partitions, so Q rope runs entirely at base 0 and K rope at base 64.
    csqk = nc.declare_dram_parameter("csqk", [128, 2 * S], BF16, False)
    masks = nc.declare_dram_parameter("masks", [128, 128], BF16, False)
    ident = nc.declare_dram_parameter("ident", [128, 128], F32, False)
    out = nc.declare_dram_parameter("out", [D, S], out_dtype, True)

    with ExitStack() as ctx:
        tc = ctx.enter_context(tile.TileContext(nc))

        consts = ctx.enter_context(tc.tile_pool(name="consts", bufs=1))
        p_mm = ctx.enter_context(tc.tile_pool(name="p_mm", bufs=5, space="PSUM"))
        p_acc = ctx.enter_context(tc.tile_pool(name="p_acc", bufs=2, space="PSUM"))
        p_cs = ctx.enter_context(tc.tile_pool(name="p_cs", bufs=1, space="PSUM"))
        p_qk = ctx.enter_context(tc.tile_pool(name="p_qk", bufs=2 * NF * NT))
        p_v = ctx.enter_context(tc.tile_pool(name="p_v", bufs=S // 128))
        p_ao = ctx.enter_context(tc.tile_pool(name="p_ao", bufs=2 * NT))
        p_tmp = ctx.enter_context(tc.tile_pool(name="p_tmp", bufs=6))
        p_w = ctx.enter_context(tc.tile_pool(name="p_w", bufs=6))
        p_wo = ctx.enter_context(tc.tile_pool(name="p_wo", bufs=2))
        p_xbf = ctx.enter_context(tc.tile_pool(name="p_xbf", bufs=4))
        p_e = ctx.enter_context(tc.tile_pool(name="p_e", bufs=7))
        p_dr = ctx.enter_context(tc.tile_pool(name="p_dr", bufs=2, space="DRAM"))
        p_sm = ctx.enter_context(tc.tile_pool(name="p_sm", bufs=3))
        p_ob = ctx.enter_context(tc.tile_pool(name="p_ob", bufs=4))

        # ---- constants ----
        csqk_sb = consts.tile([128, 2 * S], BF16, tag="csqk", name="csqk_sb")
        masks_sb = consts.tile([128, 128], BF16, tag="masks", name="masks_sb")
        identity_sb = consts.tile([128, 128], F32, tag="ident",
                                  name="identity_sb")
        ones_col = consts.tile([128, 1], BF16, tag="ones_col", name="ones_col")
        ones_row = consts.tile([1, 128], BF16, tag="ones_row", name="ones_row")
        # ones_col carries 1/SAO so the colsum reciprocal broadcasts SAO/sum
        nc.vector.memset(ones_col[:], 1.0 / SAO)
        nc.vector.memset(ones_row[:], 1.0)

        # persistent activation tiles
        # per (head, token-tile) tiles: scores of q-tile t read only the
        # token-tiles <= t, so per-tile granularity keeps the next tile's
        # RoPE writes from stalling them (deps would otherwise cover the
        # whole [128, S] tile)
        qt_sb = [[p_qk.tile([128, 512], BF16, tag="qk", name=f"qt{i}_{t}")
                  for t in range(NT)] for i in range(NF)]
        kt_sb = [[p_qk.tile([128, 512], BF16, tag="qk", name=f"kt{i}_{t}")
                  for t in range(NT)] for i in range(NF)]
        v_sb = [p_v.tile([128, F], BF16, tag="v", name=f"v{i}") for i in range(S // 128)]
        # attention out, fp8 split pair; per q-tile, head-major 512-col blocks
        ao1_t = [p_ao.tile([128, NF * DQT], F8, tag="ao", name=f"ao1_{t}")
                 for t in range(NT)]
        ao2_t = [p_ao.tile([128, NF * DQT], F8, tag="ao", name=f"ao2_{t}")
                 for t in range(NT)]
        ao1_3 = [t_[:].rearrange("p (h t) -> p h t", h=NF) for t_ in ao1_t]
        ao2_3 = [t_[:].rearrange("p (h t) -> p h t", h=NF) for t_ in ao2_t]

        # ---- DMA loads.  The HWDGE descriptor generator is ONE shared
        # device (~0.63us per DMA regardless of ring), so operands load as
        # [128, 4096] tiles (8 consecutive 128-row chunks refolded via a
        # (c p) f -> p c f pattern) - half the DMA count of per-group tiles.
        # Each tensor becomes 2 tiles = a list of 8 DoubleRow pair APs
        # [128, 2, cols] (pair pr at view [:, 2l:2l+2, :] of tile pr//4). ----
        def load_pairs(dram, name, ring, csl, pool, tag, split0=0):
            # ONE [128, 8192] tile / ONE ~1MB DMA per tensor: each HWDGE
            # ring completes DMAs serially end-to-end (~2.2us overhead +
            # transfer), so ~1MB is where the two rings saturate the
            # 344GB/s transfer device.  split0 carves a leading 2-chunk
            # sliver so the kernel's first matmul isn't gated on the rest.
            gt = pool.tile([128, 8192], F8, tag=tag, name=name)
            src = dram[:, csl].rearrange("(c p) t -> p c t", p=128)
            if split0:
                # sliver / first-half / rest: pairs 1-3 arrive ~1us sooner
                # than with a single remainder DMA
                ring.dma_start(out=gt[:, 0:split0 * 512],
                               in_=src[:, 0:split0])
                ring.dma_start(out=gt[:, split0 * 512:4096],
                               in_=src[:, split0:8])
                ring.dma_start(out=gt[:, 4096:8192], in_=src[:, 8:16])
            else:
                ring.dma_start(out=gt[:], in_=src)
            g3 = gt[:].rearrange("p (c t) -> p c t", c=16)
            return [g3[:, 2 * l:2 * l + 2, :] for l in range(8)]

        def load_x(tt):
            # tensors alternate the two HWDGE rings
            tsl = slice(tt * 512, (tt + 1) * 512)
            x1 = load_pairs(x1T, f"x1t{tt}", nc.scalar, tsl, p_xbf, "xbf")
            x2 = load_pairs(x2T, f"x2t{tt}", nc.sync, tsl, p_xbf, "xbf")
            return x1, x2

        # Tile-0 stream in strict need order.  The DMA transfer stage is ONE
        # serialized ~344GB/s device, so transfer order == readiness order;
        # the two HWDGE rings only decouple the per-engine SEQ streams (an
        # in-order SEQ with a 4-deep wait queue parks DMAs behind blocked
        # compute ops, so DMAs must be EMITTED before ops that wait).
        # Phase order inside each projection is mains -> corrB(w2) ->
        # corrA(x2), matching the delivery order below.  The leading x1/wq1
        # tiles load in a 2-chunk sliver + remainder so the first matmul
        # starts early.
        # need-ordered startup stream; halves alternate rings so the HWDGE
        # round-robin reproduces this order on the serialized transfer device
        t0 = slice(0, 512)
        x1p = load_pairs(x1T, "x1t0", nc.scalar, t0, p_xbf, "xbf", split0=2)
        wq1p = load_pairs(wq1T, "wq1", nc.sync, slice(0, F), p_w, "w",
                          split0=2)
        wk1p = load_pairs(wk1T, "wk1", nc.scalar, slice(0, F), p_w, "w")
        wq2p = load_pairs(wq2T, "wq2", nc.sync, slice(0, F), p_w, "w")
        x2p = load_pairs(x2T, "x2t0", nc.scalar, t0, p_xbf, "xbf")
        cs2 = lambda t, lo, hi: t.rearrange("p (h c) -> p h c", h=2)[:, :, lo:hi]
        nc.sync.dma_start(out=cs2(csqk_sb[:], 0, 512),
                          in_=cs2(csqk[:, :], 0, 512))
        wv1p = load_pairs(wv1T, "wv1", nc.scalar, slice(0, F), p_w, "w")
        wk2p = load_pairs(wk2T, "wk2", nc.sync, slice(0, F), p_w, "w")
        wv2p = load_pairs(wv2T, "wv2", nc.scalar, slice(0, F), p_w, "w")
        nc.sync.dma_start(out=masks_sb[:], in_=masks[:, :])
        wo1_sb = p_wo.tile([128, 4 * D], F8, tag="wo", name="wo1_sb")
        wo2_sb = p_wo.tile([128, 4 * D], F8, tag="wo", name="wo2_sb")
        wo1g3 = wo1_sb[:].rearrange("p (c d) -> p c d", c=4)
        wo2g3 = wo2_sb[:].rearrange("p (c d) -> p c d", c=4)

        # pair-AP column slicers
        def wpair(pairs, pr, ft):
            return pairs[pr][:, :, ft * 128:(ft + 1) * 128]

        def xmpair(pairs, pr):
            return pairs[pr]

        def xspair(pairs, pr, tc4):
            return pairs[pr][:, :, tc4 * 128:(tc4 + 1) * 128]

        def rope(ps, dst, ft, base, tsl):
            """RoPE: accumulator rows 0:64 = even(ve), 64:128 = odd(vo).

            The PSUM halves are first evicted with two ACT copies (ACT is
            idle through the projection phases) so the bank frees
            immediately - otherwise the next phase's PSUM allocations sit
            behind the 6-op RoPE burst on the DVE/Pool queues.  base = 0
            selects the Q rows of csqk and runs every SBUF+SBUF
            TensorTensor at partition base 0; base = 64 (K) runs at base
            64 - the hardware requires equal SBUF input bases."""
            rows = slice(base, base + 64)
            tt = tsl.start // 512
            # Evict the PSUM halves to bf16 SBUF via ACT (idle through the
            # projection phases): frees the PSUM bank after ~1.1us instead
            # of holding it through the DVE mul queue, AND makes every mul
            # an all-SBUF packed-2-byte op, which DVE runs in 4x mode
            # (~0.25 cycles/elem vs 1.0 for f32-from-PSUM).  Q ropes live
            # at partition base 0, K ropes at base 64, matching the csqk
            # rows - SBUF+SBUF TensorTensor requires equal input bases.
            ve = p_tmp.tile([128, 512], BF16, tag="rps", name="ve",
                            bufs=2)[rows, :]
            vo = p_tmp.tile([128, 512], BF16, tag="rps", name="vo",
                            bufs=2)[rows, :]
            if base == 0:
                # Q evicts ride ACT (idle mid-phase); K evicts ride DVE so
                # ACT's queue is fully drained when the next attention
                # tile's exp stream starts
                nc.scalar.copy(ve, ps[0:64, :])
                nc.scalar.copy(vo, ps[64:128, :])
            else:
                nc.vector.tensor_copy(ve, ps[0:64, :])
                nc.vector.tensor_copy(vo, ps[64:128, :])
            c = csqk_sb[rows, tsl]
            s = csqk_sb[rows, S + tsl.start:S + tsl.stop]
            t1 = p_tmp.tile([128, 512], BF16, tag="rt", name="t1",
                            bufs=4)[rows, :]
            t2 = p_tmp.tile([128, 512], BF16, tag="rt", name="t2",
                            bufs=4)[rows, :]
            nc.vector.tensor_mul(t1, ve, c)
            nc.vector.tensor_mul(t2, vo, s)
            nc.gpsimd.tensor_sub(dst[ft][tt][0:64, :], t1, t2)
            t3 = p_tmp.tile([128, 512], BF16, tag="rt", name="t3",
                            bufs=4)[rows, :]
            t4 = p_tmp.tile([128, 512], BF16, tag="rt", name="t4",
                            bufs=4)[rows, :]
            nc.vector.tensor_mul(t3, ve, s)
            nc.vector.tensor_mul(t4, vo, c)
            nc.gpsimd.tensor_add(dst[ft][tt][64:128, :], t3, t4)

        def emit_qk(tt, x1g3, x2g3):
            """Q and K projections for tile tt -> RoPE -> bf16 SBUF.

            3-term fp8 chains: main (w1,x1), corrB (w2,x1), corrA (w1,x2),
            ft-outer (weights resident for tiles 1-3)."""
            tsl = slice(tt * 512, (tt + 1) * 512)
            for w1, w2, dst, base in ((wq1p, wq2p, qt_sb, 0),
                                      (wk1p, wk2p, kt_sb, 64)):
                for ft in range(NF):
                    ps = p_mm.tile([128, 512], F32, tag="mm", name="ps")
                    for wt, xt, ph in ((w1, x1g3, 0), (w2, x1g3, 1),
                                       (w1, x2g3, 2)):
                        for pr in range(NP):
                            nc.tensor.matmul(
                                ps[:], wpair(wt, pr, ft), xmpair(xt, pr),
                                start=(ph == 0 and pr == 0),
                                stop=(ph == 2 and pr == NP - 1),
                                perf_mode=DR,
                            )
                    rope(ps, dst, ft, base, tsl)

        def emit_tile0_qkv(mid_hook=None):
            """Tile-0 QKV against the serialized DMA delivery stream.

            All 8 PSUM banks are used at once: Q ft-chains on 4 p_mm
            buffers, K ft-chains on p_acc x2 + p_cs + the 5th p_mm buffer,
            so Q and K mains interleave pair-wise and consume the wq1/wk1
            stream at delivery cadence.  Phases then follow the stream
            order (wq2, x2, wv1, wk2, wv2): Q corrB, Q corrA (close Q,
            RoPE), K corrA, V wv1-terms on Q's freed banks, K corrB (close
            K, RoPE), V x2-terms, V wv2-terms (close V)."""
            tsl = slice(0, 512)
            qb = [p_mm.tile([128, 512], F32, tag="mm", name=f"q{ft}")
                  for ft in range(NF)]
            kb = [p_acc.tile([128, 512], F32, tag="acc", name="k0"),
                  p_acc.tile([128, 512], F32, tag="acc", name="k1"),
                  p_cs.tile([128, 512], F32, tag="cs", name="k2"),
                  p_mm.tile([128, 512], F32, tag="mm", name="k3")]
            # Q mains first (x1+wq1 are the first 2MB of the stream), K
            # mains after (wk1 is the next MB and lands during Q mains)
            for pr in range(NP):
                for ft in range(NF):
                    nc.tensor.matmul(qb[ft][:], wpair(wq1p, pr, ft),
                                     xmpair(x1p, pr), start=(pr == 0),
                                     stop=False, perf_mode=DR)
            for pr in range(NP):
                for ft in range(NF):
                    nc.tensor.matmul(kb[ft][:], wpair(wk1p, pr, ft),
                                     xmpair(x1p, pr), start=(pr == 0),
                                     stop=False, perf_mode=DR)
            for pr in range(NP):        # Q corrB (wq2)
                for ft in range(NF):
                    nc.tensor.matmul(qb[ft][:], wpair(wq2p, pr, ft),
                                     xmpair(x1p, pr), start=False,
                                     stop=False, perf_mode=DR)
            for pr in range(NP):        # Q corrA (x2) - closes Q
                for ft in range(NF):
                    nc.tensor.matmul(qb[ft][:], wpair(wq1p, pr, ft),
                                     xmpair(x2p, pr), start=False,
                                     stop=(pr == NP - 1), perf_mode=DR)
            for ft in range(NF):
                rope(qb[ft], qt_sb, ft, 0, tsl)
            if mid_hook is not None:
                mid_hook()
            for pr in range(NP):        # K corrA (x2, resident)
                for ft in range(NF):
                    nc.tensor.matmul(kb[ft][:], wpair(wk1p, pr, ft),
                                     xmpair(x2p, pr), start=False,
                                     stop=False, perf_mode=DR)
            # V wv1-terms on Q's freed banks: mains then x2-corr
            vb = [p_mm.tile([128, F], F32, tag="mm", name=f"psv{i}")
                  for i in range(4)]
            for xt in (x1p, x2p):
                for pr in range(NP):
                    for tc4 in range(4):
                        nc.tensor.matmul(vb[tc4][:], xspair(xt, pr, tc4),
                                         xmpair(wv1p, pr),
                                         start=(xt is x1p and pr == 0),
                                         stop=False, perf_mode=DR)
            for pr in range(NP):        # K corrB (wk2) - closes K
                for ft in range(NF):
                    nc.tensor.matmul(kb[ft][:], wpair(wk2p, pr, ft),
                                     xmpair(x1p, pr), start=False,
                                     stop=(pr == NP - 1), perf_mode=DR)
            for ft in range(NF):
                rope(kb[ft], kt_sb, ft, 64, tsl)
            for pr in range(NP):        # V wv2-corr - closes V
                for tc4 in range(4):
                    nc.tensor.matmul(vb[tc4][:], xspair(x1p, pr, tc4),
                                     xmpair(wv2p, pr), start=False,
                                     stop=(pr == NP - 1), perf_mode=DR)
            for tc4 in range(4):
                nc.scalar.mul(v_sb[tc4][:], vb[tc4][:], V_DESCALE)

        def emit_v(tt, x1g3, x2g3):
            """V projection for tile tt (layout [t, f]), proj-phase form."""
            for tc4 in range(4):
                tch = tt * 4 + tc4
                ps = p_mm.tile([128, F], F32, tag="mm", name="psv")
                # x2 is resident before wv2 lands: corrB (x2*wv1) first
                for xt, wt, ph in ((x1g3, wv1p, 0), (x2g3, wv1p, 1),
                                   (x1g3, wv2p, 2)):
                    for pr in range(NP):
                        nc.tensor.matmul(
                            ps[:], xspair(xt, pr, tc4), xmpair(wt, pr),
                            start=(ph == 0 and pr == 0),
                            stop=(ph == 2 and pr == NP - 1),
                            perf_mode=DR,
                        )
                nc.scalar.mul(v_sb[tch][:], ps[:], V_DESCALE)

        def v_emitters(tt, x1g3, x2g3):
            """Per-matmul emitters for tile tt's V projection (filler form).
            3 * NP = 24 matmuls + 1 scaled eviction per 128-token chunk."""
            for tc4 in range(4):
                tch = tt * 4 + tc4
                ps_box = {}
                for j, (xt, wt, ph) in enumerate(
                        ((x1g3, wv1p, 0), (x2g3, wv1p, 1),
                         (x1g3, wv2p, 2))):
                    for pr in range(NP):
                        def emit(tc4=tc4, tch=tch, pr=pr, xt=xt, wt=wt,
                                 ph=ph, ps_box=ps_box):
                            if ph == 0 and pr == 0:
                                ps_box["ps"] = p_mm.tile([128, F], F32,
                                                         tag="mm", name="psv")
                            nc.tensor.matmul(
                                ps_box["ps"][:], xspair(xt, pr, tc4),
                                xmpair(wt, pr),
                                start=(ph == 0 and pr == 0),
                                stop=(ph == 2 and pr == NP - 1),
                                perf_mode=DR,
                            )
                            if ph == 2 and pr == NP - 1:
                                nc.vector.tensor_scalar_mul(
                                    v_sb[tch][:], ps_box["ps"][:], V_DESCALE)
                        yield emit

        def wo_chain_steps(wt, do):
            """Yield (lhsT, rhs, start, stop) for one output do-chain."""
            steps = []
            for i, (w3, a3) in enumerate(((wo1g3, ao1_3[wt]),
                                          (wo1g3, ao2_3[wt]),
                                          (wo2g3, ao1_3[wt]))):
                for pr in range(2):
                    steps.append((
                        w3[:, 2 * pr:2 * pr + 2, do * 128:(do + 1) * 128],
                        a3[:, 2 * pr:2 * pr + 2, :],
                        i == 0 and pr == 0,
                        i == 2 and pr == 1,
                    ))
            return steps

        def evict_out(ob_box, ps, do, wsl, act_ok=False, ring_flip=[0]):
            """Evict a closed do-chain (descaled to bf16) and, on odd do,
            DMA the 2-chain [256-row, 512] block with ONE transfer - the
            HWDGE generator is the scarce device, not DMA bandwidth.  DMAs
            alternate HWDGE rings.  act_ok puts odd-do evictions on ACT
            (only where the exp stream is done)."""
            if do % 2 == 0:
                ob_box["ob"] = p_ob.tile([128, 1024], out_dtype, tag="ob",
                                         name="ob")
            half = (do % 2) * 512
            if act_ok and do % 2 == 1:
                nc.scalar.mul(ob_box["ob"][:, half:half + 512], ps[:],
                              OUT_DESCALE)
            else:
                nc.vector.tensor_scalar_mul(
                    ob_box["ob"][:, half:half + 512], ps[:], OUT_DESCALE)
            if do % 2 == 1:
                ring = nc.sync if ring_flip[0] % 2 == 0 else nc.scalar
                ring_flip[0] += 1
                dst = out[(do - 1) * 128:(do + 1) * 128, wsl].rearrange(
                    "(c p) t -> p c t", p=128)
                ring.dma_start(out=dst, in_=ob_box["ob"][:].rearrange(
                    "p (c t) -> p c t", c=2))

        def wo_emitters(wt, alt_act=False):
            """Per-matmul emitters for tile wt's output projection."""
            wsl = slice(wt * 512, (wt + 1) * 512)
            ob_box = {}
            for do in range(ND):
                ps_box = {}
                for k, (lhsT, rhs, st, sp) in enumerate(wo_chain_steps(wt, do)):
                    def emit(do=do, k=k, lhsT=lhsT, rhs=rhs, st=st, sp=sp,
                             ps_box=ps_box):
                        if st:
                            ps_box["ps"] = p_mm.tile([128, 512], F32, tag="mm",
                                                     name="pso")
                        nc.tensor.matmul(ps_box["ps"][:], lhsT, rhs,
                                         start=st, stop=sp, perf_mode=DR)
                        if sp:
                            evict_out(ob_box, ps_box["ps"][:], do, wsl)
                    yield emit

        def emit_attention(tt, filler_iter, n_fill, pre_last_normalize=None):
            """Causal attention for q-tile tt, software-pipelined with a
            1-chunk scores lookahead and paced PE fillers."""
            qt = tt
            qsl = slice(tt * 512, (tt + 1) * 512)
            hsl_of = lambda h: slice(h * DQT, (h + 1) * DQT)
            n_kc = 4 * qt + 4
            chunks = [(h, kc) for h in range(HC) for kc in range(n_kc)]
            n_ch = len(chunks)
            st_tiles = {}

            def emit_st(h, kc):
                j = kc - 4 * qt
                qoff = 128 * j if j > 0 else 0
                st = p_mm.tile([128, DQT], F32, tag="mm", name="st")
                nc.tensor.matmul(
                    st[:, qoff:],
                    kt_sb[h][kc // 4][:, (kc % 4) * 128:(kc % 4 + 1) * 128],
                    qt_sb[h][qt][:, qoff:],
                    start=True, stop=True,
                )
                st_tiles[(h, kc)] = st

            outp_t, cs_t = {}, {}
            pv_started, cs_started = {}, {}
            pending_norm = [None]
            cs0_pending = [None]
            taken = 0
            # lookahead depth: tile 0 has 6 fillers per chunk, which already
            # cover the exp latency - depth 1 there frees a p_mm bank at
            # head boundaries.  The thinner the filler stream, the deeper
            # the ST lookahead needed to ride out ACT queue latency.
            ratio = n_fill // n_ch
            la = 1 if ratio >= 6 else (2 if ratio >= 3 else 3)
            for j0 in range(min(la, n_ch)):
                emit_st(*chunks[j0])
            for i, (h, kc) in enumerate(chunks):
                j = kc - 4 * qt
                qoff = 128 * j if j > 0 else 0
                last_head = (tt == NT - 1 and h == HC - 1)
                if kc == 0:
                    outp_t[h] = p_acc.tile([128, DQT], F32, tag="acc",
                                           name="outp")
                    pv_started[h] = False
                    cs_started[h] = False
                    cs_t[h] = p_cs.tile([128, 4], F32, tag="cs", name="cs4")
                outp = outp_t[h]

                def pv_mm(lo, hi, stop, h=h, kc=kc, outp=outp):
                    st_flag = not pv_started[h]
                    pv_started[h] = True
                    nc.tensor.matmul(
                        outp[:, lo:hi], v_sb[kc][:, h * 128:(h + 1) * 128],
                        e[:, lo:hi], start=st_flag, stop=stop,
                    )

                def cs_mm(c, stop, h=h):
                    st_flag = not cs_started[h]
                    cs_started[h] = True
                    nc.tensor.matmul(
                        cs_t[h][:, c:c + 1], e[:, c * 128:(c + 1) * 128],
                        ones_col[:], start=st_flag, stop=stop,
                    )

                st = st_tiles.pop((h, kc))
                e = p_e.tile([128, DQT], BF16, tag="e", name="e")
                nc.scalar.activation(
                    e[:, qoff:], st[:, qoff:],
                    mybir.ActivationFunctionType.Exp)
                if j >= 0:
                    nc.gpsimd.tensor_mul(
                        e[:, qoff:qoff + 128], e[:, qoff:qoff + 128],
                        masks_sb[:])
                # scores lookahead: later chunks' STs reach the PE before
                # this chunk's PV, so exp latency hides behind them
                if i + la < n_ch:
                    emit_st(*chunks[i + la])
                # paced pure-PE fillers (shifted one chunk late so the
                # attention start doesn't contend with the projection
                # phase's RoPE-pending PSUM tiles).  att(0)'s fillers are
                # V(1) whose x is still streaming in - hold them back so
                # the in-order PE isn't parked on the x(1) DMA.
                shift = 12 if tt >= 1 else -2
                want = min(n_fill, max(0, (n_fill * (i + 1)) // n_ch + shift))
                while taken < want:
                    next(filler_iter)()
                    taken += 1
                # PV + colsum.  For diagonal chunks, the open region
                # [qoff+128:) does not depend on the mask multiply - emit it
                # first so only the masked 128-block waits on the Pool hop.
                # chunk-0 colsum matmuls are deferred one chunk: the cs4
                # bank is freed by the PREVIOUS head's reciprocal, and
                # deferring the group's first touch gives that reciprocal a
                # chunk of slack before the in-order PE queue needs the slot
                # (accumulation order within the start-zeroed region is free)
                if kc == 1 and cs0_pending[0] is not None:
                    e0, j0 = cs0_pending[0]
                    cs0_pending[0] = None
                    for c in range(max(0, j0), 4):
                        st_flag = not cs_started[h]
                        cs_started[h] = True
                        nc.tensor.matmul(
                            cs_t[h][:, c:c + 1],
                            e0[:, c * 128:(c + 1) * 128],
                            ones_col[:], start=st_flag, stop=False,
                        )
                if j >= 0:
                    if qoff + 128 < DQT:
                        pv_mm(qoff + 128, DQT, False)
                    if kc == 0:
                        cs0_pending[0] = (e, j)
                    else:
                        for c in range(j + 1, 4):
                            cs_mm(c, False)
                    pv_mm(qoff, qoff + 128, kc == n_kc - 1)
                    if kc != 0:
                        cs_mm(j, j == 3)
                else:
                    pv_mm(0, DQT, False)
                    if kc == 0:
                        cs0_pending[0] = (e, j)
                    else:
                        for c in range(4):
                            cs_mm(c, False)
                if kc == n_kc - 1:
                    # normalize: reciprocal first (it unblocks the broadcast
                    # chain), accumulator eviction after, all off the PE
                    rbc = p_sm.tile([128, DQT], F32, tag="rbc", name="rbc")
                    if not last_head:
                        # transposing DRAM bounce: rdr[c, p] = rc4[p, c] =
                        # recip(q = 128c + p), so the broadcast read is a
                        # contiguous stride-0-partition AP
                        rc4 = p_sm.tile([128, 4], F32, tag="rc4", name="rc4")
                        nc.vector.reciprocal(rc4[:], cs_t[h][:])
                        rdr = p_dr.tile([4, 128], F32, tag="rdr", name="rdr")
                        nc.sync.dma_start(out=rdr[:, :].transpose([1, 0]),
                                          in_=rc4[:])
                        nc.sync.dma_start(
                            out=rbc[:],
                            in_=rdr[:, :].flatten().unsqueeze(0)
                            .to_broadcast((128, DQT)))
                        # the evict+scale wait on the bounce; deferring their
                        # EMISSION until after the next head's reciprocal
                        # keeps the parked scale from blocking the in-order
                        # DVE queue (the reciprocal must turn around fast to
                        # free the single cs4 PSUM slot)
                        if pending_norm[0] is not None:
                            pending_norm[0]()
                        def finish(h=h, outp=outp, rbc=rbc, tt=tt):
                            am = p_sm.tile([128, DQT], F32, tag="am",
                                           name="am", bufs=2)
                            if tt == 0:
                                # tile-0 heads are short: evict so the PSUM
                                # slot recycles before PV(h+2) needs it
                                outp_sb = p_sm.tile([128, DQT], F32,
                                                    tag="osb", bufs=1,
                                                    name="outp_sb")
                                nc.vector.tensor_copy(outp_sb[:], outp[:])
                                nc.vector.tensor_mul(am[:], outp_sb[:],
                                                     rbc[:])
                            else:
                                # scale straight out of the PSUM accumulator
                                # - no eviction copy; the bank's next user is
                                # a full head away, far later than the bounce
                                nc.vector.tensor_mul(am[:], outp[:], rbc[:])
                            # fp8 split of the (SAO-scaled) attention out.
                            # Both halves ride DVE: ACT must stay exp-only
                            # (the PSUM-bank WAR chain stalls the PE when
                            # exps lag) and Pool mask-only (the diagonal PV
                            # waits on it).
                            hs = hsl_of(h)
                            nc.vector.tensor_copy(ao1_t[tt][:, hs], am[:])
                            nc.vector.tensor_sub(ao2_t[tt][:, hs], am[:],
                                                 ao1_t[tt][:, hs])
                        pending_norm[0] = finish
                    else:
                        # last head sits on the critical path into the final
                        # output projection: stay on-chip.  Four tiny PE
                        # transposes assemble rcol[1, 512] from rc4's columns
                        # (adds onto the start-zeroed PSUM region), then a PE
                        # outer product broadcasts it; the first output-
                        # projection chains run as PE filler meanwhile.
                        rc4 = p_sm.tile([128, 4], F32, tag="rc4", name="rc4")
                        nc.vector.reciprocal(rc4[:], cs_t[h][:])
                        if pending_norm[0] is not None:
                            pending_norm[0]()
                            pending_norm[0] = None
                        # the WO pre-fill goes to the PE BEFORE the
                        # transposes, which must idle-wait on the DVE
                        # reciprocal otherwise
                        if pre_last_normalize is not None:
                            pre_last_normalize()
                        rcol_ps = p_mm.tile([1, DQT], F32, tag="mm",
                                            name="rcol_ps")
                        for c in range(4):
                            nc.tensor.matmul(
                                rcol_ps[0:1, c * 128:(c + 1) * 128],
                                rc4[:, c:c + 1], identity_sb[:],
                                is_transpose=True,
                                start=(c == 0), stop=(c == 3),
                            )
                        rcol_bf = p_sm.tile([1, DQT], BF16, tag="rcolbf",
                                            name="rcol_bf", bufs=1)
                        # the broadcast-chain copies ride ACT (exp-idle by
                        # now) so DVE is free for the pending head-2 norm
                        nc.scalar.copy(rcol_bf[:], rcol_ps[:])
                        rbc_ps = p_mm.tile([128, DQT], F32, tag="mm",
                                           name="rbc_ps")
                        nc.tensor.matmul(rbc_ps[:], ones_row[:], rcol_bf[:],
                                         start=True, stop=True)
                        nc.scalar.copy(rbc[:], rbc_ps[:])
                        am = p_sm.tile([128, DQT], F32, tag="am",
                                       name="am", bufs=2)
                        # the last head feeds the final output projection:
                        # keep mul -> a1 -> a2 back-to-back on DVE, no
                        # cross-engine hops on the critical chain
                        nc.vector.tensor_mul(am[:], outp[:], rbc[:])
                        hs = hsl_of(h)
                        nc.vector.tensor_copy(ao1_t[tt][:, hs], am[:])
                        nc.vector.tensor_sub(ao2_t[tt][:, hs], am[:],
                                             ao1_t[tt][:, hs])
            if pending_norm[0] is not None:
                pending_norm[0]()
                pending_norm[0] = None
            while taken < n_fill:
                next(filler_iter)()
                taken += 1

        # ================= main schedule =================
        # late-needed constants and wo tiles ride after x(1) in the stream
        nc.sync.dma_start(out=cs2(csqk_sb[:], 512, S),
                          in_=cs2(csqk[:, :], 512, S))
        nc.sync.dma_start(out=identity_sb[:], in_=ident[:, :])
        nc.sync.dma_start(out=wo1_sb[:],
                          in_=wo1T[:, :].rearrange("(c p) d -> p c d", p=128))
        nc.sync.dma_start(out=wo2_sb[:],
                          in_=wo2T[:, :].rearrange("(c p) d -> p c d", p=128))
        xh = {}
        emit_tile0_qkv(mid_hook=lambda: xh.update(x1=load_x(1)))
        x1p_1, x2p_1 = xh["x1"]
        emit_attention(0, v_emitters(1, x1p_1, x2p_1), 96)

        emit_qk(1, x1p_1, x2p_1)
        x1p_2, x2p_2 = load_x(2)
        emit_attention(1, wo_emitters(0), 96)

        emit_qk(2, x1p_2, x2p_2)
        emit_v(2, x1p_2, x2p_2)
        x1p_3, x2p_3 = load_x(3)
        # att(2) has more filler than it needs (2/chunk) while att(3) is
        # thinnest (1.5/chunk against a near-saturated ACT): shift the last
        # do-chains of WO(1) into att(3)'s filler stream
        wo1_iter = wo_emitters(1)
        emit_attention(2, wo1_iter, 96)

        emit_qk(3, x1p_3, x2p_3)
        emit_v(3, x1p_3, x2p_3)

        # last tile's output projection is split: each do-chain's pair-0
        # matmuls (heads 0-1, ready before the last head finishes) can run
        # as PE filler during the last head's normalize; only the pair-1
        # matmuls wait on the final ao
        wsl3 = slice(3 * 512, 4 * 512)
        wo3_ps = {}

        def wo3_open(do):
            ps = p_mm.tile([128, 512], F32, tag="mm", name="pso")
            wo3_ps[do] = ps
            steps = wo_chain_steps(3, do)
            for lhsT, rhs, st, sp in (steps[0], steps[2], steps[4]):
                nc.tensor.matmul(ps[:], lhsT, rhs, start=st, stop=False,
                                 perf_mode=DR)

        wo3_ob = {}

        def wo3_close(do):
            # evictions alternate DVE/ACT (ACT is exp-idle by now); out DMAs
            # pair 2 do-chains and alternate rings
            ps = wo3_ps.pop(do)
            steps = wo_chain_steps(3, do)
            for lhsT, rhs, st, sp in (steps[1], steps[3], steps[5]):
                nc.tensor.matmul(ps[:], lhsT, rhs, start=False, stop=sp,
                                 perf_mode=DR)
            evict_out(wo3_ob, ps[:], do, wsl3, act_ok=True)

        def pre_tail():
            wo3_open(0)
            wo3_open(1)
            wo3_open(2)
            wo3_open(3)

        emit_attention(3, wo_emitters(2), 96, pre_last_normalize=pre_tail)

        for do in range(4):
            wo3_close(do)
        for do in range(4, ND):
            wo3_open(do)
            wo3_close(do)

    nc.finalize()
    return nc


_ROPE_PERM_HEAD = np.concatenate([np.arange(0, HEAD_DIM, 2),
                                  np.arange(1, HEAD_DIM, 2)])


def _rope_perm(n_heads):
    return np.concatenate([h * HEAD_DIM + _ROPE_PERM_HEAD for h in range(n_heads)])


def make_masks():
    """Causal triangle: mask[kl, ql] = 1.0 if ql >= kl else 0 (bf16)."""
    import ml_dtypes
    kl = np.arange(128)[:, None]
    ql = np.arange(128)[None, :]
    return (ql >= kl).astype(np.float32).astype(ml_dtypes.bfloat16)


def _split8(a, scale):
    """fp8e4m3 two-term split of scale*a (host side)."""
    import ml_dtypes
    E4 = ml_dtypes.float8_e4m3
    a = np.asarray(a, np.float32) * np.float32(scale)
    a1 = a.astype(E4)
    a2 = (a - a1.astype(np.float32)).astype(E4)
    return np.ascontiguousarray(a1), np.ascontiguousarray(a2)


def make_in_maps(x, freqs_cos, freqs_sin, wq, wk, wv, wo,
                 D=DIM, S=SEQ, HC=HEADS_PER_CORE, n_cores=N_CORES):
    """Shard + relayout the full inputs into per-core input dicts (fp8)."""
    import ml_dtypes
    BF = ml_dtypes.bfloat16
    x = np.asarray(x, np.float32)
    B = x.shape[0]
    F = HC * HEAD_DIM
    n_groups = n_cores // B
    perm = _rope_perm(HC)
    scale = 1.0 / np.sqrt(np.float32(HEAD_DIM))
    PROJ_DESCALE = np.float32(1.0 / (SX * SW))

    cosT = np.ascontiguousarray(np.asarray(freqs_cos, np.float32).T)  # [64, S]
    sinT = np.ascontiguousarray(np.asarray(freqs_sin, np.float32).T)
    # csqk absorbs the Q/K projection descale 1/(SX*SW); layout
    # [128, 2S]: rows 0:64 = Q (cos | sin), rows 64:128 = K (cos | sin)
    csqk = np.concatenate([
        np.concatenate([cosT * scale, sinT * scale], 1) * PROJ_DESCALE,
        np.concatenate([cosT, sinT], 1) * PROJ_DESCALE,
    ], 0).astype(BF)
    masks = make_masks()

    xsplit = [_split8(x[b].T, SX) for b in range(B)]

    in_maps = []
    for i in range(n_cores):
        b, g = i // n_groups, i % n_groups
        fsl = slice(g * F, (g + 1) * F)
        wq_s = np.asarray(wq, np.float32)[fsl][perm]
        wk_s = np.asarray(wk, np.float32)[fsl][perm]
        wv_s = np.asarray(wv, np.float32)[fsl]
        wo_s = np.asarray(wo, np.float32)[:, fsl]
        wq1, wq2 = _split8(wq_s.T, SW)
        wk1, wk2 = _split8(wk_s.T, SW)
        wv1, wv2 = _split8(wv_s.T, SW)
        wo1, wo2 = _split8(wo_s.T, SW)
        in_maps.append({
            "x1T": xsplit[b][0], "x2T": xsplit[b][1],
            "wq1T": wq1, "wq2T": wq2,
            "wk1T": wk1, "wk2T": wk2,
            "wv1T": wv1, "wv2T": wv2,
            "wo1T": wo1, "wo2T": wo2,
            "csqk": csqk, "masks": masks,
            "ident": np.eye(128, dtype=np.float32),
        })
    return in_maps


_EXEC_CACHE = None


def _get_executor():
    """Build the graph once and jit-compile the 8-core SPMD executor.

    Mirrors concourse.bass2jax.run_bass_via_pjrt, but cached so repeated
    kernel() calls skip graph construction and lowering.
    """
    global _EXEC_CACHE
    if _EXEC_CACHE is not None:
        return _EXEC_CACHE

    import jax
    from jax.sharding import Mesh, PartitionSpec
    from jax.experimental.shard_map import shard_map
    from concourse import bass2jax, mybir as mb
    from concourse.bass2jax import _bass_exec_p, install_neuronx_cc_hook

    nc = build_graph()
    install_neuronx_cc_hook()
    partition_name = (nc.partition_id_tensor.name
                      if nc.partition_id_tensor else None)
    in_names, out_names, out_avals = [], [], []
    for alloc in nc.m.functions[0].allocations:
        if not isinstance(alloc, mb.MemoryLocationSet):
            continue
        name = alloc.memorylocations[0].name
        if alloc.kind == "ExternalInput":
            if name != partition_name:
                in_names.append(name)
        elif alloc.kind == "ExternalOutput":
            out_names.append(name)
            out_avals.append(jax.core.ShapedArray(
                tuple(alloc.tensor_shape), mb.dt.np(alloc.dtype)))
    n_params = len(in_names)
    n_outs = len(out_avals)
    all_in_names = list(in_names) + list(out_names)
    if partition_name is not None:
        all_in_names.append(partition_name)

    def _body(*args):
        operands = list(args)
        if partition_name is not None:
            operands.append(bass2jax.partition_id_tensor())
        outs = _bass_exec_p.bind(
            *operands,
            out_avals=tuple(out_avals),
            in_names=tuple(all_in_names),
            out_names=tuple(out_names),
            lowering_input_output_aliases=(),
            sim_require_finite=True,
            sim_require_nnan=True,
            nc=nc,
        )
        return tuple(outs)

    devices = jax.devices()[:N_CORES]
    mesh = Mesh(np.asarray(devices), ("core",))
    sharded = jax.jit(
        shard_map(_body, mesh=mesh,
                  in_specs=(PartitionSpec("core"),) * (n_params + n_outs),
                  out_specs=(PartitionSpec("core"),) * n_outs,
                  check_rep=False),
        donate_argnums=tuple(range(n_params, n_params + n_outs)),
        keep_unused=True,
    )
    _EXEC_CACHE = (sharded, in_names, out_names, out_avals, mesh)
    return _EXEC_CACHE


def run_device(in_maps):
    """Run the SPMD kernel; returns per-core output dicts."""
    import jax
    import jax.numpy as jnp
    from jax.sharding import NamedSharding, PartitionSpec

    sharded, in_names, out_names, out_avals, mesh = _get_executor()
    shard = NamedSharding(mesh, PartitionSpec("core"))
    concat_in = [
        np.concatenate([np.asarray(in_maps[c][nm]) for c in range(N_CORES)],
                       axis=0)
        for nm in in_names
    ]
    in_dev = [jax.device_put(a, shard) for a in concat_in]
    zeros = [jnp.zeros((N_CORES * av.shape[0], *av.shape[1:]), av.dtype,
                       device=shard) for av in out_avals]
    out_arrs = sharded(*in_dev, *zeros)
    return [
        {nm: np.asarray(out_arrs[i]).reshape(N_CORES, *out_avals[i].shape)[c]
         for i, nm in enumerate(out_names)}
        for c in range(N_CORES)
    ]


def kernel(x, start_pos, freqs_cos, freqs_sin, mask, wq, wk, wv, wo):
    in_maps = make_in_maps(x, freqs_cos, freqs_sin, wq, wk, wv, wo)
    results = run_device(in_maps)

    B = np.asarray(x).shape[0]
    n_groups = N_CORES // B
    out = np.empty((B, SEQ, DIM), np.float32)
    for b in range(B):
        acc = np.zeros((DIM, SEQ), np.float32)
        for g in range(n_groups):
            acc += np.asarray(results[b * n_groups + g]["out"],
                              dtype=np.float32)
        out[b] = acc.T
    return out
